# revision 4
# baseline (speedup 1.0000x reference)
"""Trainium2 Bass kernel for nn_CausalFreqMixer (causal depthwise long-conv mixer).

Math: p = x @ W + b -> [v, g1, g2] ; for each stage: v = irfft(rfft(v,4096)*Hs)[:L] * gs.

Implementation: full-DFT-as-matmul. The per-channel frequency filter is a
pointwise multiply; the DFT/IDFT along seq are channel-shared dense matrices,
so they run on the TensorEngine as [2048x2048] @ [2048x512] matmuls with the
natural [seq-partition, channel-free] layout (no transposes anywhere).

Packed-real spectrum: bins 0..2047 with Re(X[2048]) (Nyquist) packed into the
Im slot of bin 0. Forward and inverse both use the SAME two symmetric
matrices, Fc[t,k]=cos(2*pi*t*k/4096) and S0[t,k]=-sin(2*pi*t*k/4096); all
irfft scaling is folded into the host-precomputed filter spectra, and the
packed DC/Nyquist slots are handled with cheap rank-1 matmul fixups.

Sharding: 8 cores = 4 batch samples x 2 channel halves. Zero cross-core
communication; the projection weight is column-sharded to each core's
channels.
"""
import numpy as np
from contextlib import ExitStack

import concourse.bass as bass
import concourse.bacc as bacc
import concourse.tile as tile
import concourse.mybir as mybir
from concourse.bass_utils import run_bass_kernel_spmd

try:
    import ml_dtypes
    _NP_BF16 = ml_dtypes.bfloat16
except ImportError:  # pragma: no cover
    _NP_BF16 = None

# ---- problem constants (hardcoded per contract) ----
B, L, D = 4, 2048, 1024
NFFT = 2 * L
ORDER = 2
N_CORES = 8
C = D // (N_CORES // B)      # 512 channels per core
NT = L // 128                # 16 seq tiles
ND = D // 128                # 8 contraction tiles for the projection
E = (ORDER + 1) * C          # 1536 projected columns per core

# ---- tuning knobs (defaults used by kernel()) ----
MM_MODE = "bf16"             # "f32" | "f32r" | "bf16"
REPEAT = 1

_ALU = mybir.AluOpType


def _mdt(mode):
    if mode == "bf16":
        return mybir.dt.bfloat16
    if mode == "f16":
        return mybir.dt.float16
    if mode == "f32r":
        return mybir.dt.float32r
    return mybir.dt.float32


def _np_mdt(mode):
    if mode == "bf16":
        return _NP_BF16
    if mode == "f16":
        return np.float16
    return np.float32


def _emit(nc, mode, repeat):
    mdt = _mdt(mode)
    f32 = mybir.dt.float32

    def mm(out, lhsT, rhs, start, stop):
        nc.tensor.matmul(out, lhsT, rhs, start=start, stop=stop)

    xt = nc.dram_tensor("xt", [D, L], mdt, kind="ExternalInput").ap()
    w = nc.dram_tensor("w", [D, E], mdt, kind="ExternalInput").ap()
    bias = nc.dram_tensor("bias", [1, E], f32, kind="ExternalInput").ap()
    # pre-transposed on host to [m, p, j, k]: the per-m DMA reads one
    # contiguous 16*128-elem line per partition (8KB/4KB chunks) instead of
    # 16 scattered 128-elem rows -- keeps the DMA engines bandwidth-bound.
    fct = nc.dram_tensor("fct", [NT, 128, NT, 128], mdt, kind="ExternalInput").ap()
    s0t = nc.dram_tensor("s0t", [NT, 128, NT, 128], mdt, kind="ExternalInput").ap()
    altc = nc.dram_tensor("altc", [L], mdt, kind="ExternalInput").ap()
    altr = nc.dram_tensor("altr", [1, 128], mdt, kind="ExternalInput").ap()
    ha = nc.dram_tensor("ha", [ORDER, L, C], f32, kind="ExternalInput").ap()
    hb = nc.dram_tensor("hb", [ORDER, L, C], f32, kind="ExternalInput").ap()
    hd0 = nc.dram_tensor("hd0", [1, ORDER * C], f32, kind="ExternalInput").ap()
    out_d = nc.dram_tensor("out", [L, C], f32, kind="ExternalOutput").ap()

    xt_r = xt.rearrange("(kd p) l -> p kd l", p=128)
    w_r = w.rearrange("(kd p) e -> p kd e", p=128)
    altc_r = altc.rearrange("(j p) -> p j", p=128)

    with tile.TileContext(nc) as tc:
        with ExitStack() as ctx:
            consts = ctx.enter_context(tc.tile_pool(name="consts", bufs=1))
            bigs = ctx.enter_context(tc.tile_pool(name="bigs", bufs=1))
            dram = ctx.enter_context(tc.tile_pool(name="dram", bufs=1, space="DRAM"))
            psum = ctx.enter_context(tc.tile_pool(name="psum", bufs=6, space="PSUM"))
            tpool = ctx.enter_context(tc.tile_pool(name="tpool", bufs=6))

            altc_s = consts.tile([128, NT], mdt)
            nc.sync.dma_start(out=altc_s, in_=altc_r)
            altr_s = consts.tile([1, 128], mdt)
            nc.sync.dma_start(out=altr_s, in_=altr)
            hd0_s = consts.tile([1, ORDER * C], f32)
            nc.sync.dma_start(out=hd0_s, in_=hd0)
            bias_s = consts.tile([128, E], f32)
            nc.sync.dma_start(out=bias_s, in_=bias.partition_broadcast(128))

            v_buf = bigs.tile([128, NT, C], mdt)
            yr_buf = bigs.tile([128, NT, C], mdt)
            yi_buf = bigs.tile([128, NT, C], mdt)

            p_scr = dram.tile([ORDER, L, C], f32)

            for _rep in range(repeat):
                # ---------------- projection ----------------
                with tc.tile_pool(name="wproj", bufs=1) as wproj, \
                        tc.tile_pool(name="xtp", bufs=2) as xtp:
                    w_s = wproj.tile([128, ND, E], mdt)
                    for kd in range(ND):
                        nc.sync.dma_start(out=w_s[:, kd, :], in_=w_r[:, kd, :])
                    for lt in range(NT):
                        xts = xtp.tile([128, ND, 128], mdt, tag="xts")
                        nc.sync.dma_start(
                            out=xts, in_=xt_r[:, :, lt * 128:(lt + 1) * 128]
                        )
                        for ch in range(ORDER + 1):
                            ps = psum.tile([128, C], f32, tag="ps")
                            for kd in range(ND):
                                mm(ps, xts[:, kd, :], w_s[:, kd, ch * C:(ch + 1) * C],
                                   start=(kd == 0), stop=(kd == ND - 1))
                            bsl = bias_s[:, ch * C:(ch + 1) * C]
                            if ch == 0:
                                nc.vector.scalar_tensor_tensor(
                                    out=v_buf[:, lt, :], in0=ps, scalar=1.0,
                                    in1=bsl, op0=_ALU.mult, op1=_ALU.add)
                            else:
                                g = tpool.tile([128, C], f32, tag="t")
                                nc.vector.scalar_tensor_tensor(
                                    out=g, in0=ps, scalar=1.0,
                                    in1=bsl, op0=_ALU.mult, op1=_ALU.add)
                                nc.sync.dma_start(
                                    out=p_scr[ch - 1, lt * 128:(lt + 1) * 128, :],
                                    in_=g)

                # ---------------- stages ----------------
                wstage_cm = tc.tile_pool(name="wstage", bufs=3)
                wstage = wstage_cm.__enter__()
                xpool_cm = tc.tile_pool(name="xpool", bufs=3)
                xpool = xpool_cm.__enter__()
                hpool_cm = tc.tile_pool(name="hpool", bufs=3)
                hpool = hpool_cm.__enter__()
                for st in range(ORDER):
                    # forward DFT + pointwise, per bin-tile m
                    for m in range(NT):
                        wf = wstage.tile([128, NT, 128], mdt, tag="wf")
                        nc.sync.dma_start(out=wf, in_=fct[m])
                        ws = wstage.tile([128, NT, 128], mdt, tag="ws")
                        nc.sync.dma_start(out=ws, in_=s0t[m])

                        psxr = psum.tile([128, C], f32, tag="ps")
                        for j in range(NT):
                            mm(psxr, wf[:, j, :], v_buf[:, j, :],
                               start=(j == 0), stop=(j == NT - 1))
                        psxi = psum.tile([128, C], f32, tag="ps")
                        for j in range(NT - 1):
                            mm(psxi, ws[:, j, :], v_buf[:, j, :],
                               start=(j == 0), stop=False)
                        if m == 0:
                            # packed-Nyquist row: Xi[0] += sum_t (-1)^t v[t]
                            for j in range(NT):
                                mm(psxi[0:1, :], altc_s[:, j:j + 1], v_buf[:, j, :],
                                   start=False, stop=False)
                        mm(psxi, ws[:, NT - 1, :], v_buf[:, NT - 1, :],
                           start=False, stop=True)

                        xr = xpool.tile([128, C], f32, tag="x")
                        nc.scalar.copy(out=xr, in_=psxr)
                        xi = xpool.tile([128, C], f32, tag="x")
                        nc.scalar.copy(out=xi, in_=psxi)

                        hat = hpool.tile([128, C], f32, tag="ha")
                        nc.sync.dma_start(out=hat, in_=ha[st, m * 128:(m + 1) * 128, :])
                        hbt = hpool.tile([128, C], f32, tag="hb")
                        nc.sync.dma_start(out=hbt, in_=hb[st, m * 128:(m + 1) * 128, :])

                        t1 = tpool.tile([128, C], f32, tag="t")
                        nc.vector.tensor_mul(t1, xr, hat)
                        t2 = tpool.tile([128, C], f32, tag="t")
                        nc.vector.tensor_mul(t2, xi, hbt)
                        nc.vector.tensor_sub(yr_buf[:, m, :], t1, t2)
                        t3 = tpool.tile([128, C], f32, tag="t")
                        nc.vector.tensor_mul(t3, xr, hbt)
                        t4 = tpool.tile([128, C], f32, tag="t")
                        nc.vector.tensor_mul(t4, xi, hat)
                        nc.vector.tensor_add(yi_buf[:, m, :], t3, t4)
                        if m == 0:
                            # packed slot: Yi[0] = Xi[0] * ReH[Nyq] * 1/N
                            nc.vector.tensor_mul(
                                yi_buf[0:1, 0, :], xi[0:1, :],
                                hd0_s[0:1, st * C:(st + 1) * C])

                    # inverse DFT + gate, per time-tile mt
                    for mt in range(NT):
                        wfi = wstage.tile([128, NT, 128], mdt, tag="wf")
                        nc.sync.dma_start(out=wfi, in_=fct[mt])
                        wsi = wstage.tile([128, NT, 128], mdt, tag="ws")
                        nc.sync.dma_start(out=wsi, in_=s0t[mt])

                        psc = psum.tile([128, C], f32, tag="ps")
                        for j in range(NT):
                            mm(psc, wfi[:, j, :], yr_buf[:, j, :],
                               start=(j == 0), stop=False)
                        for j in range(NT):
                            mm(psc, wsi[:, j, :], yi_buf[:, j, :],
                               start=False, stop=False)
                        # packed slot contribution: conv[t] += (-1)^t * Yi[0]
                        mm(psc, altr_s[0:1, :],
                           yi_buf[0:1, 0, :], start=False, stop=True)

                        gin = tpool.tile([128, C], f32, tag="t")
                        nc.sync.dma_start(
                            out=gin, in_=p_scr[st, mt * 128:(mt + 1) * 128, :])
                        if st < ORDER - 1:
                            nc.vector.scalar_tensor_tensor(
                                out=v_buf[:, mt, :], in0=psc, scalar=1.0,
                                in1=gin, op0=_ALU.mult, op1=_ALU.mult)
                        else:
                            og = tpool.tile([128, C], f32, tag="t")
                            nc.vector.scalar_tensor_tensor(
                                out=og, in0=psc, scalar=1.0,
                                in1=gin, op0=_ALU.mult, op1=_ALU.mult)
                            nc.sync.dma_start(
                                out=out_d[mt * 128:(mt + 1) * 128, :], in_=og)
                hpool_cm.__exit__(None, None, None)
                xpool_cm.__exit__(None, None, None)
                wstage_cm.__exit__(None, None, None)


_PROGRAMS = {}


def build_program(mode=None, repeat=None):
    mode = MM_MODE if mode is None else mode
    repeat = REPEAT if repeat is None else repeat
    key = (mode, repeat)
    if key not in _PROGRAMS:
        nc = bacc.Bacc("TRN2", target_bir_lowering=False, debug=False,
                       enable_asserts=False, num_devices=N_CORES)
        _emit(nc, mode, repeat)
        nc.compile()
        _PROGRAMS[key] = nc
    return _PROGRAMS[key]


_TABLES = {}


def host_tables(mode=None):
    """Shared DFT matrices, pre-tiled as [m, j, p, k] = M[128j+p, 128m+k]."""
    mode = MM_MODE if mode is None else mode
    if mode not in _TABLES:
        npdt = _np_mdt(mode)
        t = np.arange(L, dtype=np.float64)
        ang = (2.0 * np.pi / NFFT) * np.outer(t, t)
        fc = np.cos(ang)
        s0 = -np.sin(ang)

        def tile4(mat):
            return np.ascontiguousarray(
                mat.reshape(NT, 128, NT, 128).transpose(2, 1, 0, 3).astype(npdt))

        alt = ((-1.0) ** np.arange(L))
        _TABLES[mode] = {
            "fct": tile4(fc),
            "s0t": tile4(s0),
            "altc": alt.astype(npdt),
            "altr": np.ascontiguousarray(alt[:128].reshape(1, 128).astype(npdt)),
        }
    return _TABLES[mode]


def filter_spectra(filter_time):
    """Packed, scale-folded filter spectra per stage: (A, B, d0) with
    Yr = Xr*A - Xi*B ; Yi = Xr*B + Xi*A except Yi[0] = Xi[0]*d0."""
    out = []
    for stg in range(ORDER):
        h = np.asarray(filter_time[stg, 0], dtype=np.float64)   # [L, D]
        H = np.fft.rfft(h, n=NFFT, axis=0)                       # [L+1, D]
        s = np.full((L, 1), 2.0 / NFFT)
        s[0, 0] = 1.0 / NFFT
        A = (H[:L].real * s).astype(np.float32)
        Bm = (H[:L].imag * s).astype(np.float32)
        Bm[0, :] = 0.0
        d0 = (H[L].real / NFFT).astype(np.float32)               # [D]
        out.append((A, Bm, d0))
    return out


def make_in_maps(x, proj_w, proj_b, filter_time, mode=None):
    mode = MM_MODE if mode is None else mode
    npdt = _np_mdt(mode)
    tables = host_tables(mode)
    specs = filter_spectra(filter_time)
    in_maps = []
    for core in range(N_CORES):
        b, half = divmod(core, N_CORES // B)
        c0 = half * C
        cols = np.concatenate(
            [np.arange(s * D + c0, s * D + c0 + C) for s in range(ORDER + 1)])
        ha = np.stack([specs[stg][0][:, c0:c0 + C] for stg in range(ORDER)])
        hb = np.stack([specs[stg][1][:, c0:c0 + C] for stg in range(ORDER)])
        hd0 = np.concatenate(
            [specs[stg][2][c0:c0 + C] for stg in range(ORDER)]).reshape(1, ORDER * C)
        in_maps.append({
            "xt": np.ascontiguousarray(np.asarray(x[b]).T.astype(npdt)),
            "w": np.ascontiguousarray(np.asarray(proj_w)[:, cols].astype(npdt)),
            "bias": np.ascontiguousarray(
                np.asarray(proj_b)[cols].astype(np.float32).reshape(1, E)),
            "fct": tables["fct"],
            "s0t": tables["s0t"],
            "altc": tables["altc"],
            "altr": tables["altr"],
            "ha": np.ascontiguousarray(ha.astype(np.float32)),
            "hb": np.ascontiguousarray(hb.astype(np.float32)),
            "hd0": np.ascontiguousarray(hd0.astype(np.float32)),
        })
    return in_maps


def gather_out(results):
    out = np.zeros((B, L, D), dtype=np.float32)
    for core in range(N_CORES):
        b, half = divmod(core, N_CORES // B)
        c0 = half * C
        out[b, :, c0:c0 + C] = results[core]["out"]
    return out


def kernel(x, proj_w, proj_b, filter_time):
    # Pull inputs to host numpy up front: device->host transfers must happen
    # BEFORE the bass NEFF executes (exec can leave the PJRT device in a
    # state where later transfers of pre-existing device arrays fail).
    x = np.asarray(x)
    proj_w = np.asarray(proj_w)
    proj_b = np.asarray(proj_b)
    filter_time = np.asarray(filter_time)
    nc = build_program()
    in_maps = make_in_maps(x, proj_w, proj_b, filter_time)
    res = run_bass_kernel_spmd(nc, in_maps, list(range(N_CORES)))
    return gather_out(res.results)

